# revision 58
# baseline (speedup 1.0000x reference)
"""Trainium2 Bass kernel for C = triu(A @ B), A/B upper-triangular 4096x4096 f32.

kernel(**inputs) takes FULL inputs {"A","B"} and returns the FULL output,
sharding across 8 NeuronCores via run_bass_kernel_spmd (SPMD: one program,
per-core data).

Design (v3, bf16 + chunked streams + B-sharing):
  C tiled into 128x512 supers (bi=row-block 0..31, jg=col-group 0..7);
  super (bi,jg) needs contraction over bk in [bi, 4jg+3]. The bk axis of
  each column jg is cut into LEFT-ALIGNED bands of 8 (last band is 4 for
  even jg). Work unit = "quad sweep": 4 supers with consecutive bi
  accumulate in 4 PSUM banks over ONE B stream covering a band. Partial
  results per (quad, band) are summed on the host.

  Numerics: single bf16 matmul per (row, step) -- 1 cyc/row on the PE,
  rel err ~2.9e-3 (gate 2e-2). PSUM accumulates fp32; partials evict as
  bf16 (host upconverts and accumulates).

  Per-core template (10 slots, identical instruction stream on all cores;
  quad types: F=full / S=staircase engagement, 8/4 steps, M=512-wide /
  D=diagonal width taper):
    [S8M, S4M, F8D, F8D, F8M, F8M, S8M, S4M, F4D, S4D]
  B-sharing pairs (_B_PAIRS): slots 1,3,5,7 carry no B columns; they host
  quads of the SAME (jg, band) as their partner slot (0,2,4,6) and read B
  from the partner's stream (stair pairs S8x->S4x share a band; the F8D
  and F8M slot pairs are co-located band pairs). Cuts input ~20%.

  DMA: the variable-width per-step stream is packed into ~0.4-1.4MB
  partition-major chunks, one HWDGE DMA each (large transfers ~80-97%%
  of the 358 GB/s per-core HBM limit). Slot order interleaves B-less
  (byte-light) slots between B-carrying ones so cumulative supply stays
  ahead of the PE. Evictions: vector+scalar engines alternate psum->sbuf
  casts; one/two SWDGE DMAs per slot; the final (S4D) slot packs live
  widths only and exits via HWDGE.

  Warm-up: ~10 dummy matmuls on zeroed SBUF during the initial DMA fill
  so the PE HAM clock-gate reaches 2.4 GHz before real work.

  MODE: "bf16" (default), "split3" (3x bf16 hi/lo matmuls, ~4.5e-6),
  "fp32" (exact, 4 cyc/row), or "f32r" (tf32-class, ~1.5e-4).
"""

import sys

sys.path.insert(0, "/opt/trn_rl_repo")

import numpy as np

N = 4096
N_CORES = 8
NB = N // 128
NJ = N // 512

MODE = "bf16"

# ---------------------------------------------------------------- schedule


def _enumerate_quads():
    """All real quads: (type, jg, band_a, band_b, r0).

    band [a,b] bk-range; quad rows bi in [r0, r0+3].
    """
    quads = []
    for jg in range(NJ):
        R = 4 * jg + 4
        # left-aligned bands of 8; trailing 4-band when R % 8 == 4
        bands = []
        a = 0
        while a < R:
            blen = 8 if R - a >= 8 else 4
            bands.append((a, a + blen - 1))
            a += blen
        for (a, b) in bands:
            diag = (b == R - 1)
            blen = b - a + 1
            # full quads: rows strictly above band
            for t in range(a // 4):
                if blen == 8:
                    quads.append(("F8D" if diag else "F8M", jg, a, b, 4 * t))
                else:
                    assert diag
                    quads.append(("F4D", jg, a, b, 4 * t))
            # staircase quads: rows inside the band
            if blen == 8:
                quads.append(("S8D" if diag else "S8M", jg, a, b, a))
                # lower staircase = 4-step stair over the band's last 4 bks
                quads.append(("S4D" if diag else "S4M", jg, a + 4, b, a + 4))
            else:
                quads.append(("S4D" if diag else "S4M", jg, a, b, a))
    return quads


# template slot types -> (steps, engagement, widths)
def _tmpl(ttype):
    L = 8 if "8" in ttype else 4
    stair = ttype.startswith("S")
    diag = ttype.endswith("D")
    widths = []
    for t in range(L):
        rem = L - 1 - t
        w = 512
        if diag and rem < 3:
            w = 128 * (rem + 1)
        widths.append(w)
    engage = [(0 if not stair else r) for r in range(4)]
    return L, engage, widths


# per-core slot list. Slots in _B_PAIRS are "B-sharing seconds": they ship
# only their A tracks and read the B columns from the partner slot's stream
# (both host quads of the SAME (jg, band), so the B data is identical).
_SLOT_TYPES = ["S8M", "S4M", "F8D", "F8D", "F8M", "F8M", "S8M", "S4M",
               "F4D", "S4D"]
_B_PAIRS = {1: 0, 3: 2, 5: 4, 7: 6}  # second_slot -> first_slot
# eviction DMA batches (consecutive slots share one large partition-major
# transfer; the final batch is the small S4D slot to keep the tail short)
_EVICT_BATCHES = [(0, 1, 2), (3, 4, 5), (6,), (7,), (8,), (9,)]


def _build_assignment():
    """assign[core][slot] = quad (type, jg, a, b, r0) hosted there.

    B-sharing pairs are co-located: (slot0, slot7) = one F8D band pair (or a
    same-band F4D pair), (slot3, slot8) = one F8M band pair, (slot1, slot2)
    and (slot5, slot6) = (S8x, S4x) stair pairs of the same band.
    """
    quads = _enumerate_quads()
    by_type = {}
    for q in quads:
        by_type.setdefault(q[0], []).append(q)
    for t in by_type:
        by_type[t].sort(key=lambda q: (q[1], q[2], q[4]))
    counts = {t: len(v) for t, v in by_type.items()}
    assert counts == {"F8M": 16, "F8D": 12, "F4D": 12, "S8M": 12,
                      "S8D": 4, "S4M": 12, "S4D": 8}, counts

    def pairs_of(lst):
        g = {}
        order = []
        for q in lst:
            k = (q[1], q[2])
            if k not in g:
                g[k] = []
                order.append(k)
            g[k].append(q)
        out = []
        for k in order:
            v = g[k]
            assert len(v) % 2 == 0, (k, len(v))
            for i in range(0, len(v), 2):
                out.append((v[i], v[i + 1]))
        return out

    f8m_pairs = pairs_of(by_type["F8M"])  # 8 pairs
    f8d_pairs = pairs_of(by_type["F8D"])  # 6 pairs
    f4d = by_type["F4D"]
    f4d_jg4 = [q for q in f4d if q[1] == 4][:2]
    f4d_jg6 = [q for q in f4d if q[1] == 6][:2]
    f8d_pairs += [tuple(f4d_jg4), tuple(f4d_jg6)]  # -> 8 pairs
    used = {id(q) for q in f4d_jg4 + f4d_jg6}
    f4d_rest = [q for q in f4d if id(q) not in used]  # 8 -> F4D slot
    assert len(f8m_pairs) == 8 and len(f8d_pairs) == 8 and len(f4d_rest) == 8

    # stair pairs: S4x of band (a..b) has a = band_a + 4 and partners the
    # S8x of the same band; trailing 4-bands have no S8 partner.
    s8_by_band = {}
    for q in by_type["S8M"] + by_type["S8D"]:
        s8_by_band[(q[1], q[2])] = q
    stair_pairs = []
    s4d_alone = []
    for q in by_type["S4M"] + by_type["S4D"]:
        p = s8_by_band.get((q[1], q[2] - 4))
        if p is not None:
            stair_pairs.append((p, q))
        else:
            s4d_alone.append(q)
    assert len(stair_pairs) == 16 and len(s4d_alone) == 4, (
        len(stair_pairs), len(s4d_alone))
    # mid pairs first, diag pairs last (sorted by partner type then band)
    stair_pairs.sort(key=lambda pq: (pq[0][0] != "S8M", pq[0][1], pq[0][2]))

    assign = [[None] * len(_SLOT_TYPES) for _ in range(N_CORES)]
    for c in range(N_CORES):
        assign[c][0], assign[c][1] = stair_pairs[2 * c]
        assign[c][2], assign[c][3] = f8d_pairs[c]
        assign[c][4], assign[c][5] = f8m_pairs[c]
        assign[c][6], assign[c][7] = stair_pairs[2 * c + 1]
        assign[c][8] = f4d_rest[c]
        assign[c][9] = s4d_alone[c] if c < 4 else None
    # paired slots must share (jg, band-end) so their B columns coincide
    for c in range(N_CORES):
        for s2, s1 in _B_PAIRS.items():
            qa, qb = assign[c][s1], assign[c][s2]
            assert qa is not None and qb is not None
            assert qa[1] == qb[1] and qa[3] == qb[3], (c, s1, s2, qa, qb)
    return assign


_ASSIGN = _build_assignment()
_TOTAL_STEPS = sum(_tmpl(t)[0] for t in _SLOT_TYPES)  # 60

_cache = {}


def _eff_w(w):
    if MODE == "f32r" and w < 256:
        return 256  # f32r runs at 1/4 rate below 256 cols
    return w


def _chunk_target(ci):
    """Per-chunk word budget: small first chunks so compute starts early."""
    return (160 * 1024, 384 * 1024)[ci] if ci < 2 else 704 * 1024


def _layout():
    """Variable-width per-step stream layout (template-static).

    Per step only the engaged A tracks and the live B columns are shipped.
    Element layout (au = A track unit cols, bu = B units):
      [A track 0 .. A track e-1 | B unit 0 (w cols) .. B unit bu-1]
    split3: au=256 (Ah|Al), bu=2 (Bh,Bl), bf16. fp32/f32r: au=128, bu=1, f32.

    Steps are packed into large chunks (one DMA each). Within a chunk the
    DRAM layout is partition-major [128, W_chunk]; each step occupies a
    column window.
    Returns (steps, chunks, total_words):
      steps[i]  = (e, w, au, bu, chunk_id, col_ofs, wpp)
      chunks[c] = (word_ofs, W)
    """
    au = 256 if MODE == "split3" else 128
    bu = 2 if MODE == "split3" else 1
    raw = []
    for s, ttype in enumerate(_SLOT_TYPES):
        L, engage, widths = _tmpl(ttype)
        has_b = s not in _B_PAIRS
        for t in range(L):
            e = sum(1 for r in range(4) if t >= engage[r])
            w = _eff_w(widths[t])
            wpp = au * e + (bu * w if has_b else 0)
            raw.append((s, t, e, w, au, bu, has_b, wpp))
    steps = []
    chunks = []
    word_ofs = 0
    col = 0
    for (s, t, e, w, au_, bu_, has_b, wpp) in raw:
        if col and 128 * (col + wpp) > _chunk_target(len(chunks)):
            chunks.append((word_ofs, col))
            word_ofs += 128 * col
            col = 0
        steps.append((s, t, e, w, au_, bu_, has_b, len(chunks), col, wpp))
        col += wpp
    chunks.append((word_ofs, col))
    total = word_ofs + 128 * col
    # index by (slot, t) for B-sharing partner lookup; widths must agree at
    # the aligned step (t_partner = t + L_partner - L_second)
    idx = {(s, t): rec for rec in steps for (s, t) in [(rec[0], rec[1])]}
    for s2, s1 in _B_PAIRS.items():
        L2 = _tmpl(_SLOT_TYPES[s2])[0]
        L1 = _tmpl(_SLOT_TYPES[s1])[0]
        for t in range(L2):
            r2 = idx[(s2, t)]
            r1 = idx[(s1, t + L1 - L2)]
            assert r1[3] == r2[3], (s2, t, r1[3], r2[3])  # same w
    return steps, chunks, total, idx

# ------------------------------------------------------------------ device


def _build_nc():
    import concourse.bacc as bacc
    import concourse.mybir as mybir
    import concourse.tile as tile

    f32 = mybir.dt.float32
    nc = bacc.Bacc()
    if MODE in ("split3", "bf16"):
        s_dt = mybir.dt.bfloat16
        store_dt = mybir.dt.bfloat16
    else:
        s_dt = {"fp32": mybir.dt.float32, "f32r": mybir.dt.float32r}[MODE]
        store_dt = mybir.dt.float32
    # bf16 mode evicts partials in bf16 (halves output DMA; host upconverts)
    cp_dt = mybir.dt.bfloat16 if MODE == "bf16" else f32
    steps_layout, chunks, total_words, step_idx = _layout()
    s_in = nc.declare_dram_parameter("S", [total_words], store_dt,
                                     isOutput=False)
    # partition-major across slots: batched eviction DMAs get long
    # per-partition contiguous runs (better descriptor efficiency)
    cp = nc.declare_dram_parameter("CP", [128, len(_SLOT_TYPES) * 2048],
                                   cp_dt, isOutput=True)

    with tile.TileContext(nc) as tc:
        with (
            tc.tile_pool(name="st", bufs=1) as s_pool,
            tc.tile_pool(name="co", bufs=1) as c_pool,
            tc.tile_pool(name="ps", bufs=2, space="PSUM") as ps_pool,
        ):
            # PE warm-up: dummy matmuls on zeroed SBUF spanning the initial
            # DMA wait so HAM un-throttles (1.2->2.4 GHz) before real work.
            # Small [128,128] tile: memset completes early and N=128 matmuls
            # (~107ns cold) end right as the first input chunk lands.
            wz = s_pool.tile([128, 128], s_dt, tag="wz", name="wz")
            nc.vector.memset(wz[:], 0)
            wu = ps_pool.tile([128, 512], f32, tag="p0", name="wu")
            for i in range(22):
                nc.tensor.matmul(wu[:, :128], lhsT=wz[:], rhs=wz[:],
                                 start=True, stop=True)
            # one big input DMA per chunk; steps slice column windows
            ch_tiles = []
            for ci, (ofs, W) in enumerate(chunks):
                src = s_in[ofs:ofs + 128 * W] \
                    .rearrange("(p w) -> p w", p=128).bitcast(s_dt)
                ch = s_pool.tile([128, W], s_dt, tag=f"ch{ci}",
                                 name=f"ch_{ci}")
                nc.sync.dma_start(out=ch[:], in_=src)
                ch_tiles.append(ch)
            cursor = 0
            batch_i = 0
            cb = None
            cb_base = 0
            for s, ttype in enumerate(_SLOT_TYPES):
                L, engage, widths = _tmpl(ttype)
                ps = [
                    ps_pool.tile([128, 512], f32, tag=f"p{r}",
                                 name=f"ps_{s}_{r}")
                    for r in range(4)
                ]
                for t in range(L):
                    _s, _t, e, w, au, bu, has_b, ci, col, wpp = \
                        steps_layout[cursor]
                    oc = 512 - w
                    st = ch_tiles[ci][:, col:col + wpp]
                    if has_b:
                        b_tile, b_ofs = st, au * e
                    else:
                        s1 = _B_PAIRS[s]
                        L1 = _tmpl(_SLOT_TYPES[s1])[0]
                        r1 = step_idx[(s1, t + L1 - L)]
                        e1, ci1, col1 = r1[2], r1[7], r1[8]
                        b_tile = ch_tiles[ci1][:, col1:col1 + r1[9]]
                        b_ofs = au * e1
                    for r in range(4):
                        if t < engage[r]:
                            continue
                        first = (t == engage[r])
                        last = (t == L - 1)
                        if MODE == "split3":
                            ah = st[:, au * r:au * r + 128]
                            al = st[:, au * r + 128:au * (r + 1)]
                            bh = b_tile[:, b_ofs:b_ofs + w]
                            bl = b_tile[:, b_ofs + w:b_ofs + 2 * w]
                            nc.tensor.matmul(ps[r][:, oc:], lhsT=ah, rhs=bh,
                                             start=first, stop=False)
                            nc.tensor.matmul(ps[r][:, oc:], lhsT=al, rhs=bh,
                                             start=False, stop=False)
                            nc.tensor.matmul(ps[r][:, oc:], lhsT=ah, rhs=bl,
                                             start=False, stop=last)
                        else:
                            nc.tensor.matmul(
                                ps[r][:, oc:],
                                lhsT=st[:, au * r:au * (r + 1)],
                                rhs=b_tile[:, b_ofs:b_ofs + w],
                                start=first, stop=last,
                            )
                    cursor += 1
                if s == _EVICT_BATCHES[batch_i][0]:
                    blen = len(_EVICT_BATCHES[batch_i])
                    cb = c_pool.tile([128, 2048 * blen], cp_dt,
                                     tag=f"cb{batch_i}", name=f"cb_{batch_i}")
                    cb_base = s
                local = 2048 * (s - cb_base)
                if ttype == "S4D":
                    # live widths per row are 512,384,256,128 -> pack tight
                    ofs = local
                    for r in range(4):
                        wv = 512 - 128 * r
                        dst = cb[:, ofs:ofs + wv]
                        src = ps[r][:, 512 - wv:]
                        if r % 2 == 0:
                            nc.vector.tensor_copy(dst, src)
                        else:
                            nc.scalar.copy(dst, src)
                        ofs += wv
                else:
                    ofs = local + 2048
                    for r in range(4):
                        dst = cb[:, local + 512 * r:local + 512 * (r + 1)]
                        if r % 2 == 0:
                            nc.vector.tensor_copy(dst, ps[r][:])
                        else:
                            nc.scalar.copy(dst, ps[r][:])
                if s == _EVICT_BATCHES[batch_i][-1]:
                    lo = 2048 * cb_base
                    nc.gpsimd.dma_start(out=cp[:, lo:lo + ofs],
                                        in_=cb[:, :ofs])
                    batch_i += 1
            assert cursor == _TOTAL_STEPS
    nc.finalize()
    return nc


def get_nc():
    key = ("nc", MODE)
    if key not in _cache:
        _cache[key] = _build_nc()
    return _cache[key]


# ------------------------------------------------------------------- host


def _make_blocks(A, B):
    """Mode-specific block views for packing."""
    A4 = A.reshape(NB, 128, NB, 128).transpose(0, 2, 3, 1)
    B4 = B.reshape(NB, 128, NJ, 512).transpose(0, 2, 1, 3)
    if MODE == "bf16":
        import ml_dtypes

        bf = ml_dtypes.bfloat16
        return {"A": [A4.astype(bf)], "B": [B4.astype(bf)], "dtype": bf}
    if MODE != "split3":
        return {"A": [A4], "B": [B4], "dtype": np.float32}
    import ml_dtypes

    bf = ml_dtypes.bfloat16
    A4h = A4.astype(bf)
    A4l = (A4 - A4h.astype(np.float32)).astype(bf)
    B4h = B4.astype(bf)
    B4l = (B4 - B4h.astype(np.float32)).astype(bf)
    return {"A": [A4h, A4l], "B": [B4h, B4l], "dtype": bf}


def _pack_core(c, blocks):
    """Flat variable-width S stream for core c (layout per _layout()).

    A blocks are transposed ([p,m] = A[128bi+m, 128bk+p]); B blocks are
    128x512 (only the live [oc:] columns are shipped).
    """
    steps_layout, chunks, total_words, _ = _layout()
    chs = [np.zeros((128, W), dtype=blocks["dtype"]) for _, W in chunks]
    cursor = 0
    for s, ttype in enumerate(_SLOT_TYPES):
        L, engage, widths = _tmpl(ttype)
        q = _ASSIGN[c][s]
        if q is None:  # ghost slot: leave zeros
            cursor += L
            continue
        qtype, jg, a, b, r0 = q
        base = b - L + 1  # bk at template step 0 (right-aligned hosting)
        for t in range(L):
            _s, _t, e, w, au, bu, has_b, ci, col, wpp = steps_layout[cursor]
            bk = base + t
            row = chs[ci][:, col:col + wpp]
            oc = 512 - w
            if has_b and bk >= a:
                for h in range(bu):
                    row[:, au * e + w * h:au * e + w * (h + 1)] = \
                        blocks["B"][h][bk, jg][:, oc:]
            for r in range(e):
                bi = r0 + r
                if bk >= a and bk >= bi:
                    for h in range(len(blocks["A"])):
                        row[:, au * r + 128 * h:au * r + 128 * (h + 1)] = \
                            blocks["A"][h][bi, bk]
            cursor += 1
    return np.concatenate([ch.reshape(-1) for ch in chs])


def _out_np_dtype():
    if MODE == "bf16":
        import ml_dtypes

        return ml_dtypes.bfloat16
    return np.float32


def _get_runner():
    """Build (once per process/MODE) a cached jitted SPMD executable.

    Mirrors bass2jax.run_bass_via_pjrt's multi-core path, but reuses the
    compiled executable across kernel() calls.
    """
    key = ("runner", MODE)
    if key in _cache:
        return _cache[key]
    import jax
    from jax.sharding import Mesh, PartitionSpec
    from jax.experimental.shard_map import shard_map
    from concourse import bass2jax, mybir

    nc = get_nc()
    bass2jax.install_neuronx_cc_hook()
    partition_name = (nc.partition_id_tensor.name
                      if nc.partition_id_tensor else None)
    out_shape = (128, len(_SLOT_TYPES) * 2048)
    out_aval = jax.core.ShapedArray(out_shape, _out_np_dtype())
    in_names = ["S", "CP"]
    if partition_name is not None:
        in_names.append(partition_name)

    def _body(s_arr, zeros):
        operands = [s_arr, zeros]
        if partition_name is not None:
            operands.append(bass2jax.partition_id_tensor())
        outs = bass2jax._bass_exec_p.bind(
            *operands, out_avals=(out_aval,), in_names=tuple(in_names),
            out_names=("CP",), lowering_input_output_aliases=(),
            sim_require_finite=True, sim_require_nnan=True, nc=nc)
        return outs[0]

    devices = jax.devices()[:N_CORES]
    mesh = Mesh(np.asarray(devices), ("core",))
    sharded = jax.jit(
        shard_map(_body, mesh=mesh,
                  in_specs=(PartitionSpec("core"),) * 2,
                  out_specs=PartitionSpec("core"), check_rep=False),
        donate_argnums=(1,), keep_unused=True)
    _cache[key] = sharded
    return sharded


def kernel(A: np.ndarray, B: np.ndarray) -> np.ndarray:
    A = np.asarray(A, dtype=np.float32)
    B = np.asarray(B, dtype=np.float32)

    blocks = _make_blocks(A, B)
    s_all = np.concatenate([_pack_core(c, blocks) for c in range(N_CORES)],
                           axis=0)
    zeros = np.zeros((N_CORES * 128, len(_SLOT_TYPES) * 2048),
                     _out_np_dtype())
    runner = _get_runner()
    out = np.asarray(runner(s_all, zeros))
    per_core = out.reshape(N_CORES, 128, len(_SLOT_TYPES) * 2048)
    if per_core.dtype != np.float32:
        per_core = per_core.astype(np.float32)

    C = np.zeros((N, N), dtype=np.float32)
    for c in range(N_CORES):
        cpk = per_core[c]
        for s, ttype in enumerate(_SLOT_TYPES):
            q = _ASSIGN[c][s]
            if q is None:
                continue
            qtype, jg, a, b, r0 = q
            if ttype == "S4D":
                ofs = 2048 * s
                for r in range(4):
                    bi = r0 + r
                    wv = 512 - 128 * r
                    C[128 * bi:128 * (bi + 1),
                      512 * jg + 128 * r:512 * (jg + 1)] += \
                        cpk[:, ofs:ofs + wv]
                    ofs += wv
                continue
            for r in range(4):
                bi = r0 + r
                blk = cpk[:, 2048 * s + 512 * r:2048 * s + 512 * (r + 1)]
                # written psum region starts at the track's start width
                L, engage, widths = _tmpl(ttype)
                w0 = widths[engage[r]]
                if MODE == "f32r" and w0 < 256:
                    w0 = 256
                lo = 512 - w0
                C[128 * bi:128 * (bi + 1),
                  512 * jg + lo:512 * (jg + 1)] += blk[:, lo:]
    return C


def _make_in_maps(A, B):
    A = np.asarray(A, dtype=np.float32)
    B = np.asarray(B, dtype=np.float32)
    blocks = _make_blocks(A, B)
    return [{"S": _pack_core(c, blocks)} for c in range(N_CORES)]



# revision 59
# speedup vs baseline: 1.0012x; 1.0012x over previous
"""Trainium2 Bass kernel for C = triu(A @ B), A/B upper-triangular 4096x4096 f32.

kernel(**inputs) takes FULL inputs {"A","B"} and returns the FULL output,
sharding across 8 NeuronCores via run_bass_kernel_spmd (SPMD: one program,
per-core data).

Design (v3, bf16 + chunked streams + B-sharing):
  C tiled into 128x512 supers (bi=row-block 0..31, jg=col-group 0..7);
  super (bi,jg) needs contraction over bk in [bi, 4jg+3]. The bk axis of
  each column jg is cut into LEFT-ALIGNED bands of 8 (last band is 4 for
  even jg). Work unit = "quad sweep": 4 supers with consecutive bi
  accumulate in 4 PSUM banks over ONE B stream covering a band. Partial
  results per (quad, band) are summed on the host.

  Numerics: single bf16 matmul per (row, step) -- 1 cyc/row on the PE,
  rel err ~2.9e-3 (gate 2e-2). PSUM accumulates fp32; partials evict as
  bf16 (host upconverts and accumulates).

  Per-core template (10 slots, identical instruction stream on all cores;
  quad types: F=full / S=staircase engagement, 8/4 steps, M=512-wide /
  D=diagonal width taper):
    [S8M, S4M, F8D, F8D, F8M, F8M, S8M, S4M, F4D, S4D]
  B-sharing pairs (_B_PAIRS): slots 1,3,5,7 carry no B columns; they host
  quads of the SAME (jg, band) as their partner slot (0,2,4,6) and read B
  from the partner's stream (stair pairs S8x->S4x share a band; the F8D
  and F8M slot pairs are co-located band pairs). Cuts input ~20%.

  DMA: the variable-width per-step stream is packed into ~0.4-1.4MB
  partition-major chunks, one HWDGE DMA each (large transfers ~80-97%%
  of the 358 GB/s per-core HBM limit). Slot order interleaves B-less
  (byte-light) slots between B-carrying ones so cumulative supply stays
  ahead of the PE. Evictions: vector+scalar engines alternate psum->sbuf
  casts; one/two SWDGE DMAs per slot; the final (S4D) slot packs live
  widths only and exits via HWDGE.

  Warm-up: ~10 dummy matmuls on zeroed SBUF during the initial DMA fill
  so the PE HAM clock-gate reaches 2.4 GHz before real work.

  MODE: "bf16" (default), "split3" (3x bf16 hi/lo matmuls, ~4.5e-6),
  "fp32" (exact, 4 cyc/row), or "f32r" (tf32-class, ~1.5e-4).
"""

import sys

sys.path.insert(0, "/opt/trn_rl_repo")

import numpy as np

N = 4096
N_CORES = 8
NB = N // 128
NJ = N // 512

MODE = "bf16"

# ---------------------------------------------------------------- schedule


def _enumerate_quads():
    """All real quads: (type, jg, band_a, band_b, r0).

    band [a,b] bk-range; quad rows bi in [r0, r0+3].
    """
    quads = []
    for jg in range(NJ):
        R = 4 * jg + 4
        # left-aligned bands of 8; trailing 4-band when R % 8 == 4
        bands = []
        a = 0
        while a < R:
            blen = 8 if R - a >= 8 else 4
            bands.append((a, a + blen - 1))
            a += blen
        for (a, b) in bands:
            diag = (b == R - 1)
            blen = b - a + 1
            # full quads: rows strictly above band
            for t in range(a // 4):
                if blen == 8:
                    quads.append(("F8D" if diag else "F8M", jg, a, b, 4 * t))
                else:
                    assert diag
                    quads.append(("F4D", jg, a, b, 4 * t))
            # staircase quads: rows inside the band
            if blen == 8:
                quads.append(("S8D" if diag else "S8M", jg, a, b, a))
                # lower staircase = 4-step stair over the band's last 4 bks
                quads.append(("S4D" if diag else "S4M", jg, a + 4, b, a + 4))
            else:
                quads.append(("S4D" if diag else "S4M", jg, a, b, a))
    return quads


# template slot types -> (steps, engagement, widths)
def _tmpl(ttype):
    L = 8 if "8" in ttype else 4
    stair = ttype.startswith("S")
    diag = ttype.endswith("D")
    widths = []
    for t in range(L):
        rem = L - 1 - t
        w = 512
        if diag and rem < 3:
            w = 128 * (rem + 1)
        widths.append(w)
    engage = [(0 if not stair else r) for r in range(4)]
    return L, engage, widths


# per-core slot list. Slots in _B_PAIRS are "B-sharing seconds": they ship
# only their A tracks and read the B columns from the partner slot's stream
# (both host quads of the SAME (jg, band), so the B data is identical).
_SLOT_TYPES = ["S8M", "S4M", "F8D", "F8D", "F8M", "F8M", "S8M", "S4M",
               "F4D", "S4D"]
_B_PAIRS = {1: 0, 3: 2, 5: 4, 7: 6}  # second_slot -> first_slot
# eviction DMA batches (consecutive slots share one large partition-major
# transfer; the final batch is the small S4D slot to keep the tail short)
_EVICT_BATCHES = [(0, 1, 2), (3, 4, 5), (6,), (7,), (8,), (9,)]


def _build_assignment():
    """assign[core][slot] = quad (type, jg, a, b, r0) hosted there.

    B-sharing pairs are co-located: (slot0, slot7) = one F8D band pair (or a
    same-band F4D pair), (slot3, slot8) = one F8M band pair, (slot1, slot2)
    and (slot5, slot6) = (S8x, S4x) stair pairs of the same band.
    """
    quads = _enumerate_quads()
    by_type = {}
    for q in quads:
        by_type.setdefault(q[0], []).append(q)
    for t in by_type:
        by_type[t].sort(key=lambda q: (q[1], q[2], q[4]))
    counts = {t: len(v) for t, v in by_type.items()}
    assert counts == {"F8M": 16, "F8D": 12, "F4D": 12, "S8M": 12,
                      "S8D": 4, "S4M": 12, "S4D": 8}, counts

    def pairs_of(lst):
        g = {}
        order = []
        for q in lst:
            k = (q[1], q[2])
            if k not in g:
                g[k] = []
                order.append(k)
            g[k].append(q)
        out = []
        for k in order:
            v = g[k]
            assert len(v) % 2 == 0, (k, len(v))
            for i in range(0, len(v), 2):
                out.append((v[i], v[i + 1]))
        return out

    f8m_pairs = pairs_of(by_type["F8M"])  # 8 pairs
    f8d_pairs = pairs_of(by_type["F8D"])  # 6 pairs
    f4d = by_type["F4D"]
    f4d_jg4 = [q for q in f4d if q[1] == 4][:2]
    f4d_jg6 = [q for q in f4d if q[1] == 6][:2]
    f8d_pairs += [tuple(f4d_jg4), tuple(f4d_jg6)]  # -> 8 pairs
    used = {id(q) for q in f4d_jg4 + f4d_jg6}
    f4d_rest = [q for q in f4d if id(q) not in used]  # 8 -> F4D slot
    assert len(f8m_pairs) == 8 and len(f8d_pairs) == 8 and len(f4d_rest) == 8

    # stair pairs: S4x of band (a..b) has a = band_a + 4 and partners the
    # S8x of the same band; trailing 4-bands have no S8 partner.
    s8_by_band = {}
    for q in by_type["S8M"] + by_type["S8D"]:
        s8_by_band[(q[1], q[2])] = q
    stair_pairs = []
    s4d_alone = []
    for q in by_type["S4M"] + by_type["S4D"]:
        p = s8_by_band.get((q[1], q[2] - 4))
        if p is not None:
            stair_pairs.append((p, q))
        else:
            s4d_alone.append(q)
    assert len(stair_pairs) == 16 and len(s4d_alone) == 4, (
        len(stair_pairs), len(s4d_alone))
    # mid pairs first, diag pairs last (sorted by partner type then band)
    stair_pairs.sort(key=lambda pq: (pq[0][0] != "S8M", pq[0][1], pq[0][2]))

    assign = [[None] * len(_SLOT_TYPES) for _ in range(N_CORES)]
    for c in range(N_CORES):
        assign[c][0], assign[c][1] = stair_pairs[2 * c]
        assign[c][2], assign[c][3] = f8d_pairs[c]
        assign[c][4], assign[c][5] = f8m_pairs[c]
        assign[c][6], assign[c][7] = stair_pairs[2 * c + 1]
        assign[c][8] = f4d_rest[c]
        assign[c][9] = s4d_alone[c] if c < 4 else None
    # paired slots must share (jg, band-end) so their B columns coincide
    for c in range(N_CORES):
        for s2, s1 in _B_PAIRS.items():
            qa, qb = assign[c][s1], assign[c][s2]
            assert qa is not None and qb is not None
            assert qa[1] == qb[1] and qa[3] == qb[3], (c, s1, s2, qa, qb)
    return assign


_ASSIGN = _build_assignment()
_TOTAL_STEPS = sum(_tmpl(t)[0] for t in _SLOT_TYPES)  # 60

_cache = {}


def _eff_w(w):
    if MODE == "f32r" and w < 256:
        return 256  # f32r runs at 1/4 rate below 256 cols
    return w


def _chunk_target(ci):
    """Per-chunk word budget: small first chunks so compute starts early."""
    return (160 * 1024, 384 * 1024)[ci] if ci < 2 else 704 * 1024


def _layout():
    """Variable-width per-step stream layout (template-static).

    Per step only the engaged A tracks and the live B columns are shipped.
    Element layout (au = A track unit cols, bu = B units):
      [A track 0 .. A track e-1 | B unit 0 (w cols) .. B unit bu-1]
    split3: au=256 (Ah|Al), bu=2 (Bh,Bl), bf16. fp32/f32r: au=128, bu=1, f32.

    Steps are packed into large chunks (one DMA each). Within a chunk the
    DRAM layout is partition-major [128, W_chunk]; each step occupies a
    column window.
    Returns (steps, chunks, total_words):
      steps[i]  = (e, w, au, bu, chunk_id, col_ofs, wpp)
      chunks[c] = (word_ofs, W)
    """
    au = 256 if MODE == "split3" else 128
    bu = 2 if MODE == "split3" else 1
    raw = []
    for s, ttype in enumerate(_SLOT_TYPES):
        L, engage, widths = _tmpl(ttype)
        has_b = s not in _B_PAIRS
        for t in range(L):
            e = sum(1 for r in range(4) if t >= engage[r])
            w = _eff_w(widths[t])
            wpp = au * e + (bu * w if has_b else 0)
            raw.append((s, t, e, w, au, bu, has_b, wpp))
    steps = []
    chunks = []
    word_ofs = 0
    col = 0
    for (s, t, e, w, au_, bu_, has_b, wpp) in raw:
        if col and 128 * (col + wpp) > _chunk_target(len(chunks)):
            chunks.append((word_ofs, col))
            word_ofs += 128 * col
            col = 0
        steps.append((s, t, e, w, au_, bu_, has_b, len(chunks), col, wpp))
        col += wpp
    chunks.append((word_ofs, col))
    total = word_ofs + 128 * col
    # index by (slot, t) for B-sharing partner lookup; widths must agree at
    # the aligned step (t_partner = t + L_partner - L_second)
    idx = {(s, t): rec for rec in steps for (s, t) in [(rec[0], rec[1])]}
    for s2, s1 in _B_PAIRS.items():
        L2 = _tmpl(_SLOT_TYPES[s2])[0]
        L1 = _tmpl(_SLOT_TYPES[s1])[0]
        for t in range(L2):
            r2 = idx[(s2, t)]
            r1 = idx[(s1, t + L1 - L2)]
            assert r1[3] == r2[3], (s2, t, r1[3], r2[3])  # same w
    return steps, chunks, total, idx

# ------------------------------------------------------------------ device


def _build_nc():
    import concourse.bacc as bacc
    import concourse.mybir as mybir
    import concourse.tile as tile

    f32 = mybir.dt.float32
    nc = bacc.Bacc()
    if MODE in ("split3", "bf16"):
        s_dt = mybir.dt.bfloat16
        store_dt = mybir.dt.bfloat16
    else:
        s_dt = {"fp32": mybir.dt.float32, "f32r": mybir.dt.float32r}[MODE]
        store_dt = mybir.dt.float32
    # bf16 mode evicts partials in bf16 (halves output DMA; host upconverts)
    cp_dt = mybir.dt.bfloat16 if MODE == "bf16" else f32
    steps_layout, chunks, total_words, step_idx = _layout()
    s_in = nc.declare_dram_parameter("S", [total_words], store_dt,
                                     isOutput=False)
    # partition-major across slots: batched eviction DMAs get long
    # per-partition contiguous runs (better descriptor efficiency)
    cp = nc.declare_dram_parameter("CP", [128, len(_SLOT_TYPES) * 2048],
                                   cp_dt, isOutput=True)

    with tile.TileContext(nc) as tc:
        with (
            tc.tile_pool(name="st", bufs=1) as s_pool,
            tc.tile_pool(name="co", bufs=1) as c_pool,
            tc.tile_pool(name="ps", bufs=2, space="PSUM") as ps_pool,
        ):
            # PE warm-up: dummy matmuls on zeroed SBUF spanning the initial
            # DMA wait so HAM un-throttles (1.2->2.4 GHz) before real work.
            # Small [128,128] tile: memset completes early and N=128 matmuls
            # (~107ns cold) end right as the first input chunk lands.
            wz = s_pool.tile([128, 128], s_dt, tag="wz", name="wz")
            nc.vector.memset(wz[:], 0)
            wu = ps_pool.tile([128, 512], f32, tag="p0", name="wu")
            for i in range(22):
                nc.tensor.matmul(wu[:, :128], lhsT=wz[:], rhs=wz[:],
                                 start=True, stop=True)
            # one big input DMA per chunk; steps slice column windows
            ch_tiles = []
            for ci, (ofs, W) in enumerate(chunks):
                src = s_in[ofs:ofs + 128 * W] \
                    .rearrange("(p w) -> p w", p=128).bitcast(s_dt)
                ch = s_pool.tile([128, W], s_dt, tag=f"ch{ci}",
                                 name=f"ch_{ci}")
                nc.sync.dma_start(out=ch[:], in_=src)
                ch_tiles.append(ch)
            cursor = 0
            batch_i = 0
            cb = None
            cb_base = 0
            for s, ttype in enumerate(_SLOT_TYPES):
                L, engage, widths = _tmpl(ttype)
                ps = [
                    ps_pool.tile([128, 512], f32, tag=f"p{r}",
                                 name=f"ps_{s}_{r}")
                    for r in range(4)
                ]
                for t in range(L):
                    _s, _t, e, w, au, bu, has_b, ci, col, wpp = \
                        steps_layout[cursor]
                    oc = 512 - w
                    st = ch_tiles[ci][:, col:col + wpp]
                    if has_b:
                        b_tile, b_ofs = st, au * e
                    else:
                        s1 = _B_PAIRS[s]
                        L1 = _tmpl(_SLOT_TYPES[s1])[0]
                        r1 = step_idx[(s1, t + L1 - L)]
                        e1, ci1, col1 = r1[2], r1[7], r1[8]
                        b_tile = ch_tiles[ci1][:, col1:col1 + r1[9]]
                        b_ofs = au * e1
                    for r in range(4):
                        if t < engage[r]:
                            continue
                        first = (t == engage[r])
                        last = (t == L - 1)
                        if MODE == "split3":
                            ah = st[:, au * r:au * r + 128]
                            al = st[:, au * r + 128:au * (r + 1)]
                            bh = b_tile[:, b_ofs:b_ofs + w]
                            bl = b_tile[:, b_ofs + w:b_ofs + 2 * w]
                            nc.tensor.matmul(ps[r][:, oc:], lhsT=ah, rhs=bh,
                                             start=first, stop=False)
                            nc.tensor.matmul(ps[r][:, oc:], lhsT=al, rhs=bh,
                                             start=False, stop=False)
                            nc.tensor.matmul(ps[r][:, oc:], lhsT=ah, rhs=bl,
                                             start=False, stop=last)
                        else:
                            nc.tensor.matmul(
                                ps[r][:, oc:],
                                lhsT=st[:, au * r:au * (r + 1)],
                                rhs=b_tile[:, b_ofs:b_ofs + w],
                                start=first, stop=last,
                            )
                    cursor += 1
                base = 2048 * s
                c_t = c_pool.tile([128, 2048], cp_dt, tag=f"c{s}",
                                  name=f"c_{s}")
                if ttype == "S4D":
                    # live widths per row are 512,384,256,128 -> pack tight
                    ofs = 0
                    for r in range(4):
                        wv = 512 - 128 * r
                        dst = c_t[:, ofs:ofs + wv]
                        src = ps[r][:, 512 - wv:]
                        if r % 2 == 0:
                            nc.vector.tensor_copy(dst, src)
                        else:
                            nc.scalar.copy(dst, src)
                        ofs += wv
                    nc.gpsimd.dma_start(out=cp[:, base:base + ofs],
                                        in_=c_t[:, :ofs])
                else:
                    for r in range(4):
                        dst = c_t[:, 512 * r:512 * (r + 1)]
                        if r % 2 == 0:
                            nc.vector.tensor_copy(dst, ps[r][:])
                        else:
                            nc.scalar.copy(dst, ps[r][:])
                        if r % 2 == 1:
                            h = r // 2
                            nc.gpsimd.dma_start(
                                out=cp[:, base + 1024 * h:
                                       base + 1024 * (h + 1)],
                                in_=c_t[:, 1024 * h:1024 * (h + 1)])
            assert cursor == _TOTAL_STEPS
    nc.finalize()
    return nc


def get_nc():
    key = ("nc", MODE)
    if key not in _cache:
        _cache[key] = _build_nc()
    return _cache[key]


# ------------------------------------------------------------------- host


def _make_blocks(A, B):
    """Mode-specific block views for packing."""
    A4 = A.reshape(NB, 128, NB, 128).transpose(0, 2, 3, 1)
    B4 = B.reshape(NB, 128, NJ, 512).transpose(0, 2, 1, 3)
    if MODE == "bf16":
        import ml_dtypes

        bf = ml_dtypes.bfloat16
        return {"A": [A4.astype(bf)], "B": [B4.astype(bf)], "dtype": bf}
    if MODE != "split3":
        return {"A": [A4], "B": [B4], "dtype": np.float32}
    import ml_dtypes

    bf = ml_dtypes.bfloat16
    A4h = A4.astype(bf)
    A4l = (A4 - A4h.astype(np.float32)).astype(bf)
    B4h = B4.astype(bf)
    B4l = (B4 - B4h.astype(np.float32)).astype(bf)
    return {"A": [A4h, A4l], "B": [B4h, B4l], "dtype": bf}


def _pack_core(c, blocks):
    """Flat variable-width S stream for core c (layout per _layout()).

    A blocks are transposed ([p,m] = A[128bi+m, 128bk+p]); B blocks are
    128x512 (only the live [oc:] columns are shipped).
    """
    steps_layout, chunks, total_words, _ = _layout()
    chs = [np.zeros((128, W), dtype=blocks["dtype"]) for _, W in chunks]
    cursor = 0
    for s, ttype in enumerate(_SLOT_TYPES):
        L, engage, widths = _tmpl(ttype)
        q = _ASSIGN[c][s]
        if q is None:  # ghost slot: leave zeros
            cursor += L
            continue
        qtype, jg, a, b, r0 = q
        base = b - L + 1  # bk at template step 0 (right-aligned hosting)
        for t in range(L):
            _s, _t, e, w, au, bu, has_b, ci, col, wpp = steps_layout[cursor]
            bk = base + t
            row = chs[ci][:, col:col + wpp]
            oc = 512 - w
            if has_b and bk >= a:
                for h in range(bu):
                    row[:, au * e + w * h:au * e + w * (h + 1)] = \
                        blocks["B"][h][bk, jg][:, oc:]
            for r in range(e):
                bi = r0 + r
                if bk >= a and bk >= bi:
                    for h in range(len(blocks["A"])):
                        row[:, au * r + 128 * h:au * r + 128 * (h + 1)] = \
                            blocks["A"][h][bi, bk]
            cursor += 1
    return np.concatenate([ch.reshape(-1) for ch in chs])


def _out_np_dtype():
    if MODE == "bf16":
        import ml_dtypes

        return ml_dtypes.bfloat16
    return np.float32


def _get_runner():
    """Build (once per process/MODE) a cached jitted SPMD executable.

    Mirrors bass2jax.run_bass_via_pjrt's multi-core path, but reuses the
    compiled executable across kernel() calls.
    """
    key = ("runner", MODE)
    if key in _cache:
        return _cache[key]
    import jax
    from jax.sharding import Mesh, PartitionSpec
    from jax.experimental.shard_map import shard_map
    from concourse import bass2jax, mybir

    nc = get_nc()
    bass2jax.install_neuronx_cc_hook()
    partition_name = (nc.partition_id_tensor.name
                      if nc.partition_id_tensor else None)
    out_shape = (128, len(_SLOT_TYPES) * 2048)
    out_aval = jax.core.ShapedArray(out_shape, _out_np_dtype())
    in_names = ["S", "CP"]
    if partition_name is not None:
        in_names.append(partition_name)

    def _body(s_arr, zeros):
        operands = [s_arr, zeros]
        if partition_name is not None:
            operands.append(bass2jax.partition_id_tensor())
        outs = bass2jax._bass_exec_p.bind(
            *operands, out_avals=(out_aval,), in_names=tuple(in_names),
            out_names=("CP",), lowering_input_output_aliases=(),
            sim_require_finite=True, sim_require_nnan=True, nc=nc)
        return outs[0]

    devices = jax.devices()[:N_CORES]
    mesh = Mesh(np.asarray(devices), ("core",))
    sharded = jax.jit(
        shard_map(_body, mesh=mesh,
                  in_specs=(PartitionSpec("core"),) * 2,
                  out_specs=PartitionSpec("core"), check_rep=False),
        donate_argnums=(1,), keep_unused=True)
    _cache[key] = sharded
    return sharded


def kernel(A: np.ndarray, B: np.ndarray) -> np.ndarray:
    A = np.asarray(A, dtype=np.float32)
    B = np.asarray(B, dtype=np.float32)

    blocks = _make_blocks(A, B)
    s_all = np.concatenate([_pack_core(c, blocks) for c in range(N_CORES)],
                           axis=0)
    zeros = np.zeros((N_CORES * 128, len(_SLOT_TYPES) * 2048),
                     _out_np_dtype())
    runner = _get_runner()
    out = np.asarray(runner(s_all, zeros))
    per_core = out.reshape(N_CORES, 128, len(_SLOT_TYPES) * 2048)
    if per_core.dtype != np.float32:
        per_core = per_core.astype(np.float32)

    C = np.zeros((N, N), dtype=np.float32)
    for c in range(N_CORES):
        cpk = per_core[c]
        for s, ttype in enumerate(_SLOT_TYPES):
            q = _ASSIGN[c][s]
            if q is None:
                continue
            qtype, jg, a, b, r0 = q
            if ttype == "S4D":
                ofs = 2048 * s
                for r in range(4):
                    bi = r0 + r
                    wv = 512 - 128 * r
                    C[128 * bi:128 * (bi + 1),
                      512 * jg + 128 * r:512 * (jg + 1)] += \
                        cpk[:, ofs:ofs + wv]
                    ofs += wv
                continue
            for r in range(4):
                bi = r0 + r
                blk = cpk[:, 2048 * s + 512 * r:2048 * s + 512 * (r + 1)]
                # written psum region starts at the track's start width
                L, engage, widths = _tmpl(ttype)
                w0 = widths[engage[r]]
                if MODE == "f32r" and w0 < 256:
                    w0 = 256
                lo = 512 - w0
                C[128 * bi:128 * (bi + 1),
                  512 * jg + lo:512 * (jg + 1)] += blk[:, lo:]
    return C


def _make_in_maps(A, B):
    A = np.asarray(A, dtype=np.float32)
    B = np.asarray(B, dtype=np.float32)
    blocks = _make_blocks(A, B)
    return [{"S": _pack_core(c, blocks)} for c in range(N_CORES)]



# revision 60
# speedup vs baseline: 1.0072x; 1.0060x over previous
"""Trainium2 Bass kernel for C = triu(A @ B), A/B upper-triangular 4096x4096 f32.

kernel(**inputs) takes FULL inputs {"A","B"} and returns the FULL output,
sharding across 8 NeuronCores via run_bass_kernel_spmd (SPMD: one program,
per-core data).

Design (v3, bf16 + chunked streams + B-sharing):
  C tiled into 128x512 supers (bi=row-block 0..31, jg=col-group 0..7);
  super (bi,jg) needs contraction over bk in [bi, 4jg+3]. The bk axis of
  each column jg is cut into LEFT-ALIGNED bands of 8 (last band is 4 for
  even jg). Work unit = "quad sweep": 4 supers with consecutive bi
  accumulate in 4 PSUM banks over ONE B stream covering a band. Partial
  results per (quad, band) are summed on the host.

  Numerics: single bf16 matmul per (row, step) -- 1 cyc/row on the PE,
  rel err ~2.9e-3 (gate 2e-2). PSUM accumulates fp32; partials evict as
  bf16 (host upconverts and accumulates).

  Per-core template (10 slots, identical instruction stream on all cores;
  quad types: F=full / S=staircase engagement, 8/4 steps, M=512-wide /
  D=diagonal width taper):
    [S8M, S4M, F8D, F8D, F8M, F8M, S8M, S4M, F4D, S4D]
  B-sharing pairs (_B_PAIRS): slots 1,3,5,7 carry no B columns; they host
  quads of the SAME (jg, band) as their partner slot (0,2,4,6) and read B
  from the partner's stream (stair pairs S8x->S4x share a band; the F8D
  and F8M slot pairs are co-located band pairs). Cuts input ~20%.

  DMA: the variable-width per-step stream is packed into ~0.4-1.4MB
  partition-major chunks, one HWDGE DMA each (large transfers ~80-97%%
  of the 358 GB/s per-core HBM limit). Slot order interleaves B-less
  (byte-light) slots between B-carrying ones so cumulative supply stays
  ahead of the PE. Evictions: vector+scalar engines alternate psum->sbuf
  casts; one/two SWDGE DMAs per slot; the final (S4D) slot packs live
  widths only and exits via HWDGE.

  Warm-up: ~10 dummy matmuls on zeroed SBUF during the initial DMA fill
  so the PE HAM clock-gate reaches 2.4 GHz before real work.

  MODE: "bf16" (default), "split3" (3x bf16 hi/lo matmuls, ~4.5e-6),
  "fp32" (exact, 4 cyc/row), or "f32r" (tf32-class, ~1.5e-4).
"""

import sys

sys.path.insert(0, "/opt/trn_rl_repo")

import numpy as np

N = 4096
N_CORES = 8
NB = N // 128
NJ = N // 512

MODE = "bf16"

# ---------------------------------------------------------------- schedule


def _enumerate_quads():
    """All real quads: (type, jg, band_a, band_b, r0).

    band [a,b] bk-range; quad rows bi in [r0, r0+3].
    """
    quads = []
    for jg in range(NJ):
        R = 4 * jg + 4
        # left-aligned bands of 8; trailing 4-band when R % 8 == 4
        bands = []
        a = 0
        while a < R:
            blen = 8 if R - a >= 8 else 4
            bands.append((a, a + blen - 1))
            a += blen
        for (a, b) in bands:
            diag = (b == R - 1)
            blen = b - a + 1
            # full quads: rows strictly above band
            for t in range(a // 4):
                if blen == 8:
                    quads.append(("F8D" if diag else "F8M", jg, a, b, 4 * t))
                else:
                    assert diag
                    quads.append(("F4D", jg, a, b, 4 * t))
            # staircase quads: rows inside the band
            if blen == 8:
                quads.append(("S8D" if diag else "S8M", jg, a, b, a))
                # lower staircase = 4-step stair over the band's last 4 bks
                quads.append(("S4D" if diag else "S4M", jg, a + 4, b, a + 4))
            else:
                quads.append(("S4D" if diag else "S4M", jg, a, b, a))
    return quads


# template slot types -> (steps, engagement, widths)
def _tmpl(ttype):
    L = 8 if "8" in ttype else 4
    stair = ttype.startswith("S")
    diag = ttype.endswith("D")
    widths = []
    for t in range(L):
        rem = L - 1 - t
        w = 512
        if diag and rem < 3:
            w = 128 * (rem + 1)
        widths.append(w)
    engage = [(0 if not stair else r) for r in range(4)]
    return L, engage, widths


# per-core slot list. Slots in _B_PAIRS are "B-sharing seconds": they ship
# only their A tracks and read the B columns from the partner slot's stream
# (both host quads of the SAME (jg, band), so the B data is identical).
_SLOT_TYPES = ["S8M", "S4M", "F8D", "F8D", "F8M", "F8M", "S8M", "S4M",
               "F4D", "S4D"]
_B_PAIRS = {1: 0, 3: 2, 5: 4, 7: 6}  # second_slot -> first_slot
# eviction DMA batches (consecutive slots share one large partition-major
# transfer; the final batch is the small S4D slot to keep the tail short)
_EVICT_BATCHES = [(0, 1, 2), (3, 4, 5), (6,), (7,), (8,), (9,)]


def _build_assignment():
    """assign[core][slot] = quad (type, jg, a, b, r0) hosted there.

    B-sharing pairs are co-located: (slot0, slot7) = one F8D band pair (or a
    same-band F4D pair), (slot3, slot8) = one F8M band pair, (slot1, slot2)
    and (slot5, slot6) = (S8x, S4x) stair pairs of the same band.
    """
    quads = _enumerate_quads()
    by_type = {}
    for q in quads:
        by_type.setdefault(q[0], []).append(q)
    for t in by_type:
        by_type[t].sort(key=lambda q: (q[1], q[2], q[4]))
    counts = {t: len(v) for t, v in by_type.items()}
    assert counts == {"F8M": 16, "F8D": 12, "F4D": 12, "S8M": 12,
                      "S8D": 4, "S4M": 12, "S4D": 8}, counts

    def pairs_of(lst):
        g = {}
        order = []
        for q in lst:
            k = (q[1], q[2])
            if k not in g:
                g[k] = []
                order.append(k)
            g[k].append(q)
        out = []
        for k in order:
            v = g[k]
            assert len(v) % 2 == 0, (k, len(v))
            for i in range(0, len(v), 2):
                out.append((v[i], v[i + 1]))
        return out

    f8m_pairs = pairs_of(by_type["F8M"])  # 8 pairs
    f8d_pairs = pairs_of(by_type["F8D"])  # 6 pairs
    f4d = by_type["F4D"]
    f4d_jg4 = [q for q in f4d if q[1] == 4][:2]
    f4d_jg6 = [q for q in f4d if q[1] == 6][:2]
    f8d_pairs += [tuple(f4d_jg4), tuple(f4d_jg6)]  # -> 8 pairs
    used = {id(q) for q in f4d_jg4 + f4d_jg6}
    f4d_rest = [q for q in f4d if id(q) not in used]  # 8 -> F4D slot
    assert len(f8m_pairs) == 8 and len(f8d_pairs) == 8 and len(f4d_rest) == 8

    # stair pairs: S4x of band (a..b) has a = band_a + 4 and partners the
    # S8x of the same band; trailing 4-bands have no S8 partner.
    s8_by_band = {}
    for q in by_type["S8M"] + by_type["S8D"]:
        s8_by_band[(q[1], q[2])] = q
    stair_pairs = []
    s4d_alone = []
    for q in by_type["S4M"] + by_type["S4D"]:
        p = s8_by_band.get((q[1], q[2] - 4))
        if p is not None:
            stair_pairs.append((p, q))
        else:
            s4d_alone.append(q)
    assert len(stair_pairs) == 16 and len(s4d_alone) == 4, (
        len(stair_pairs), len(s4d_alone))
    # mid pairs first, diag pairs last (sorted by partner type then band)
    stair_pairs.sort(key=lambda pq: (pq[0][0] != "S8M", pq[0][1], pq[0][2]))

    assign = [[None] * len(_SLOT_TYPES) for _ in range(N_CORES)]
    for c in range(N_CORES):
        assign[c][0], assign[c][1] = stair_pairs[2 * c]
        assign[c][2], assign[c][3] = f8d_pairs[c]
        assign[c][4], assign[c][5] = f8m_pairs[c]
        assign[c][6], assign[c][7] = stair_pairs[2 * c + 1]
        assign[c][8] = f4d_rest[c]
        assign[c][9] = s4d_alone[c] if c < 4 else None
    # paired slots must share (jg, band-end) so their B columns coincide
    for c in range(N_CORES):
        for s2, s1 in _B_PAIRS.items():
            qa, qb = assign[c][s1], assign[c][s2]
            assert qa is not None and qb is not None
            assert qa[1] == qb[1] and qa[3] == qb[3], (c, s1, s2, qa, qb)
    return assign


_ASSIGN = _build_assignment()
_TOTAL_STEPS = sum(_tmpl(t)[0] for t in _SLOT_TYPES)  # 60

_cache = {}


def _eff_w(w):
    if MODE == "f32r" and w < 256:
        return 256  # f32r runs at 1/4 rate below 256 cols
    return w


def _chunk_target(ci):
    """Per-chunk word budget: small first chunks so compute starts early."""
    return (160 * 1024, 384 * 1024)[ci] if ci < 2 else 704 * 1024


def _layout():
    """Variable-width per-step stream layout (template-static).

    Per step only the engaged A tracks and the live B columns are shipped.
    Element layout (au = A track unit cols, bu = B units):
      [A track 0 .. A track e-1 | B unit 0 (w cols) .. B unit bu-1]
    split3: au=256 (Ah|Al), bu=2 (Bh,Bl), bf16. fp32/f32r: au=128, bu=1, f32.

    Steps are packed into large chunks (one DMA each). Within a chunk the
    DRAM layout is partition-major [128, W_chunk]; each step occupies a
    column window.
    Returns (steps, chunks, total_words):
      steps[i]  = (e, w, au, bu, chunk_id, col_ofs, wpp)
      chunks[c] = (word_ofs, W)
    """
    au = 256 if MODE == "split3" else 128
    bu = 2 if MODE == "split3" else 1
    raw = []
    for s, ttype in enumerate(_SLOT_TYPES):
        L, engage, widths = _tmpl(ttype)
        has_b = s not in _B_PAIRS
        for t in range(L):
            e = sum(1 for r in range(4) if t >= engage[r])
            w = _eff_w(widths[t])
            wpp = au * e + (bu * w if has_b else 0)
            raw.append((s, t, e, w, au, bu, has_b, wpp))
    steps = []
    chunks = []
    word_ofs = 0
    col = 0
    for (s, t, e, w, au_, bu_, has_b, wpp) in raw:
        if col and 128 * (col + wpp) > _chunk_target(len(chunks)):
            chunks.append((word_ofs, col))
            word_ofs += 128 * col
            col = 0
        steps.append((s, t, e, w, au_, bu_, has_b, len(chunks), col, wpp))
        col += wpp
    chunks.append((word_ofs, col))
    total = word_ofs + 128 * col
    # index by (slot, t) for B-sharing partner lookup; widths must agree at
    # the aligned step (t_partner = t + L_partner - L_second)
    idx = {(s, t): rec for rec in steps for (s, t) in [(rec[0], rec[1])]}
    for s2, s1 in _B_PAIRS.items():
        L2 = _tmpl(_SLOT_TYPES[s2])[0]
        L1 = _tmpl(_SLOT_TYPES[s1])[0]
        for t in range(L2):
            r2 = idx[(s2, t)]
            r1 = idx[(s1, t + L1 - L2)]
            assert r1[3] == r2[3], (s2, t, r1[3], r2[3])  # same w
    return steps, chunks, total, idx

# ------------------------------------------------------------------ device


def _build_nc():
    import concourse.bacc as bacc
    import concourse.mybir as mybir
    import concourse.tile as tile

    f32 = mybir.dt.float32
    nc = bacc.Bacc()
    if MODE in ("split3", "bf16"):
        s_dt = mybir.dt.bfloat16
        store_dt = mybir.dt.bfloat16
    else:
        s_dt = {"fp32": mybir.dt.float32, "f32r": mybir.dt.float32r}[MODE]
        store_dt = mybir.dt.float32
    # bf16 mode evicts partials in bf16 (halves output DMA; host upconverts)
    cp_dt = mybir.dt.bfloat16 if MODE == "bf16" else f32
    steps_layout, chunks, total_words, step_idx = _layout()
    s_in = nc.declare_dram_parameter("S", [total_words], store_dt,
                                     isOutput=False)
    # partition-major across slots: batched eviction DMAs get long
    # per-partition contiguous runs (better descriptor efficiency)
    cp = nc.declare_dram_parameter("CP", [128, len(_SLOT_TYPES) * 2048],
                                   cp_dt, isOutput=True)

    with tile.TileContext(nc) as tc:
        with (
            tc.tile_pool(name="st", bufs=1) as s_pool,
            tc.tile_pool(name="co", bufs=1) as c_pool,
            tc.tile_pool(name="ps", bufs=2, space="PSUM") as ps_pool,
        ):
            # PE warm-up: dummy matmuls on zeroed SBUF spanning the initial
            # DMA wait so HAM un-throttles (1.2->2.4 GHz) before real work.
            wz = s_pool.tile([128, 512], s_dt, tag="wz", name="wz")
            nc.vector.memset(wz[:], 0)
            wu = ps_pool.tile([128, 512], f32, tag="p0", name="wu")
            for i in range(14):
                nc.tensor.matmul(wu[:], lhsT=wz[:, :128], rhs=wz[:],
                                 start=True, stop=True)
            # one big input DMA per chunk; steps slice column windows
            ch_tiles = []
            for ci, (ofs, W) in enumerate(chunks):
                src = s_in[ofs:ofs + 128 * W] \
                    .rearrange("(p w) -> p w", p=128).bitcast(s_dt)
                ch = s_pool.tile([128, W], s_dt, tag=f"ch{ci}",
                                 name=f"ch_{ci}")
                nc.sync.dma_start(out=ch[:], in_=src)
                ch_tiles.append(ch)
            cursor = 0
            batch_i = 0
            cb = None
            cb_base = 0
            for s, ttype in enumerate(_SLOT_TYPES):
                L, engage, widths = _tmpl(ttype)
                ps = [
                    ps_pool.tile([128, 512], f32, tag=f"p{r}",
                                 name=f"ps_{s}_{r}")
                    for r in range(4)
                ]
                for t in range(L):
                    _s, _t, e, w, au, bu, has_b, ci, col, wpp = \
                        steps_layout[cursor]
                    oc = 512 - w
                    st = ch_tiles[ci][:, col:col + wpp]
                    if has_b:
                        b_tile, b_ofs = st, au * e
                    else:
                        s1 = _B_PAIRS[s]
                        L1 = _tmpl(_SLOT_TYPES[s1])[0]
                        r1 = step_idx[(s1, t + L1 - L)]
                        e1, ci1, col1 = r1[2], r1[7], r1[8]
                        b_tile = ch_tiles[ci1][:, col1:col1 + r1[9]]
                        b_ofs = au * e1
                    for r in range(4):
                        if t < engage[r]:
                            continue
                        first = (t == engage[r])
                        last = (t == L - 1)
                        if MODE == "split3":
                            ah = st[:, au * r:au * r + 128]
                            al = st[:, au * r + 128:au * (r + 1)]
                            bh = b_tile[:, b_ofs:b_ofs + w]
                            bl = b_tile[:, b_ofs + w:b_ofs + 2 * w]
                            nc.tensor.matmul(ps[r][:, oc:], lhsT=ah, rhs=bh,
                                             start=first, stop=False)
                            nc.tensor.matmul(ps[r][:, oc:], lhsT=al, rhs=bh,
                                             start=False, stop=False)
                            nc.tensor.matmul(ps[r][:, oc:], lhsT=ah, rhs=bl,
                                             start=False, stop=last)
                        else:
                            nc.tensor.matmul(
                                ps[r][:, oc:],
                                lhsT=st[:, au * r:au * (r + 1)],
                                rhs=b_tile[:, b_ofs:b_ofs + w],
                                start=first, stop=last,
                            )
                    cursor += 1
                base = 2048 * s
                c_t = c_pool.tile([128, 2048], cp_dt, tag=f"c{s}",
                                  name=f"c_{s}")
                if ttype == "S4D":
                    # live widths per row are 512,384,256,128 -> pack tight
                    ofs = 0
                    for r in range(4):
                        wv = 512 - 128 * r
                        dst = c_t[:, ofs:ofs + wv]
                        src = ps[r][:, 512 - wv:]
                        if r % 2 == 0:
                            nc.vector.tensor_copy(dst, src)
                        else:
                            nc.scalar.copy(dst, src)
                        ofs += wv
                    nc.gpsimd.dma_start(out=cp[:, base:base + ofs],
                                        in_=c_t[:, :ofs])
                else:
                    for r in range(4):
                        dst = c_t[:, 512 * r:512 * (r + 1)]
                        if r % 2 == 0:
                            nc.vector.tensor_copy(dst, ps[r][:])
                        else:
                            nc.scalar.copy(dst, ps[r][:])
                        if r % 2 == 1:
                            h = r // 2
                            nc.gpsimd.dma_start(
                                out=cp[:, base + 1024 * h:
                                       base + 1024 * (h + 1)],
                                in_=c_t[:, 1024 * h:1024 * (h + 1)])
            assert cursor == _TOTAL_STEPS
    nc.finalize()
    return nc


def get_nc():
    key = ("nc", MODE)
    if key not in _cache:
        _cache[key] = _build_nc()
    return _cache[key]


# ------------------------------------------------------------------- host


def _make_blocks(A, B):
    """Mode-specific block views for packing."""
    A4 = A.reshape(NB, 128, NB, 128).transpose(0, 2, 3, 1)
    B4 = B.reshape(NB, 128, NJ, 512).transpose(0, 2, 1, 3)
    if MODE == "bf16":
        import ml_dtypes

        bf = ml_dtypes.bfloat16
        return {"A": [A4.astype(bf)], "B": [B4.astype(bf)], "dtype": bf}
    if MODE != "split3":
        return {"A": [A4], "B": [B4], "dtype": np.float32}
    import ml_dtypes

    bf = ml_dtypes.bfloat16
    A4h = A4.astype(bf)
    A4l = (A4 - A4h.astype(np.float32)).astype(bf)
    B4h = B4.astype(bf)
    B4l = (B4 - B4h.astype(np.float32)).astype(bf)
    return {"A": [A4h, A4l], "B": [B4h, B4l], "dtype": bf}


def _pack_core(c, blocks):
    """Flat variable-width S stream for core c (layout per _layout()).

    A blocks are transposed ([p,m] = A[128bi+m, 128bk+p]); B blocks are
    128x512 (only the live [oc:] columns are shipped).
    """
    steps_layout, chunks, total_words, _ = _layout()
    chs = [np.zeros((128, W), dtype=blocks["dtype"]) for _, W in chunks]
    cursor = 0
    for s, ttype in enumerate(_SLOT_TYPES):
        L, engage, widths = _tmpl(ttype)
        q = _ASSIGN[c][s]
        if q is None:  # ghost slot: leave zeros
            cursor += L
            continue
        qtype, jg, a, b, r0 = q
        base = b - L + 1  # bk at template step 0 (right-aligned hosting)
        for t in range(L):
            _s, _t, e, w, au, bu, has_b, ci, col, wpp = steps_layout[cursor]
            bk = base + t
            row = chs[ci][:, col:col + wpp]
            oc = 512 - w
            if has_b and bk >= a:
                for h in range(bu):
                    row[:, au * e + w * h:au * e + w * (h + 1)] = \
                        blocks["B"][h][bk, jg][:, oc:]
            for r in range(e):
                bi = r0 + r
                if bk >= a and bk >= bi:
                    for h in range(len(blocks["A"])):
                        row[:, au * r + 128 * h:au * r + 128 * (h + 1)] = \
                            blocks["A"][h][bi, bk]
            cursor += 1
    return np.concatenate([ch.reshape(-1) for ch in chs])


def _out_np_dtype():
    if MODE == "bf16":
        import ml_dtypes

        return ml_dtypes.bfloat16
    return np.float32


def _get_runner():
    """Build (once per process/MODE) a cached jitted SPMD executable.

    Mirrors bass2jax.run_bass_via_pjrt's multi-core path, but reuses the
    compiled executable across kernel() calls.
    """
    key = ("runner", MODE)
    if key in _cache:
        return _cache[key]
    import jax
    from jax.sharding import Mesh, PartitionSpec
    from jax.experimental.shard_map import shard_map
    from concourse import bass2jax, mybir

    nc = get_nc()
    bass2jax.install_neuronx_cc_hook()
    partition_name = (nc.partition_id_tensor.name
                      if nc.partition_id_tensor else None)
    out_shape = (128, len(_SLOT_TYPES) * 2048)
    out_aval = jax.core.ShapedArray(out_shape, _out_np_dtype())
    in_names = ["S", "CP"]
    if partition_name is not None:
        in_names.append(partition_name)

    def _body(s_arr, zeros):
        operands = [s_arr, zeros]
        if partition_name is not None:
            operands.append(bass2jax.partition_id_tensor())
        outs = bass2jax._bass_exec_p.bind(
            *operands, out_avals=(out_aval,), in_names=tuple(in_names),
            out_names=("CP",), lowering_input_output_aliases=(),
            sim_require_finite=True, sim_require_nnan=True, nc=nc)
        return outs[0]

    devices = jax.devices()[:N_CORES]
    mesh = Mesh(np.asarray(devices), ("core",))
    sharded = jax.jit(
        shard_map(_body, mesh=mesh,
                  in_specs=(PartitionSpec("core"),) * 2,
                  out_specs=PartitionSpec("core"), check_rep=False),
        donate_argnums=(1,), keep_unused=True)
    _cache[key] = sharded
    return sharded


def kernel(A: np.ndarray, B: np.ndarray) -> np.ndarray:
    A = np.asarray(A, dtype=np.float32)
    B = np.asarray(B, dtype=np.float32)

    blocks = _make_blocks(A, B)
    s_all = np.concatenate([_pack_core(c, blocks) for c in range(N_CORES)],
                           axis=0)
    zeros = np.zeros((N_CORES * 128, len(_SLOT_TYPES) * 2048),
                     _out_np_dtype())
    runner = _get_runner()
    out = np.asarray(runner(s_all, zeros))
    per_core = out.reshape(N_CORES, 128, len(_SLOT_TYPES) * 2048)
    if per_core.dtype != np.float32:
        per_core = per_core.astype(np.float32)

    C = np.zeros((N, N), dtype=np.float32)
    for c in range(N_CORES):
        cpk = per_core[c]
        for s, ttype in enumerate(_SLOT_TYPES):
            q = _ASSIGN[c][s]
            if q is None:
                continue
            qtype, jg, a, b, r0 = q
            if ttype == "S4D":
                ofs = 2048 * s
                for r in range(4):
                    bi = r0 + r
                    wv = 512 - 128 * r
                    C[128 * bi:128 * (bi + 1),
                      512 * jg + 128 * r:512 * (jg + 1)] += \
                        cpk[:, ofs:ofs + wv]
                    ofs += wv
                continue
            for r in range(4):
                bi = r0 + r
                blk = cpk[:, 2048 * s + 512 * r:2048 * s + 512 * (r + 1)]
                # written psum region starts at the track's start width
                L, engage, widths = _tmpl(ttype)
                w0 = widths[engage[r]]
                if MODE == "f32r" and w0 < 256:
                    w0 = 256
                lo = 512 - w0
                C[128 * bi:128 * (bi + 1),
                  512 * jg + lo:512 * (jg + 1)] += blk[:, lo:]
    return C


def _make_in_maps(A, B):
    A = np.asarray(A, dtype=np.float32)
    B = np.asarray(B, dtype=np.float32)
    blocks = _make_blocks(A, B)
    return [{"S": _pack_core(c, blocks)} for c in range(N_CORES)]



# revision 62
# speedup vs baseline: 1.0639x; 1.0563x over previous
"""Trainium2 Bass kernel for C = triu(A @ B), A/B upper-triangular 4096x4096 f32.

kernel(**inputs) takes FULL inputs {"A","B"} and returns the FULL output,
sharding across 8 NeuronCores via run_bass_kernel_spmd (SPMD: one program,
per-core data).

Design (v3, bf16 + chunked streams + B-sharing):
  C tiled into 128x512 supers (bi=row-block 0..31, jg=col-group 0..7);
  super (bi,jg) needs contraction over bk in [bi, 4jg+3]. The bk axis of
  each column jg is cut into LEFT-ALIGNED bands of 8 (last band is 4 for
  even jg). Work unit = "quad sweep": 4 supers with consecutive bi
  accumulate in 4 PSUM banks over ONE B stream covering a band. Partial
  results per (quad, band) are summed on the host.

  Numerics: single bf16 matmul per (row, step) -- 1 cyc/row on the PE,
  rel err ~2.9e-3 (gate 2e-2). PSUM accumulates fp32; partials evict as
  bf16 (host upconverts and accumulates).

  Per-core template (10 slots, identical instruction stream on all cores;
  quad types: F=full / S=staircase engagement, 8/4 steps, M=512-wide /
  D=diagonal width taper):
    [S8M, S4M, F8D, F8D, F8M, F8M, S8M, S4M, F4D, S4D]
  B-sharing pairs (_B_PAIRS): slots 1,3,5,7 carry no B columns; they host
  quads of the SAME (jg, band) as their partner slot (0,2,4,6) and read B
  from the partner's stream (stair pairs S8x->S4x share a band; the F8D
  and F8M slot pairs are co-located band pairs). Cuts input ~20%.

  DMA: the variable-width per-step stream is packed into ~0.4-1.4MB
  partition-major chunks, one HWDGE DMA each (large transfers ~80-97%%
  of the 358 GB/s per-core HBM limit). Slot order interleaves B-less
  (byte-light) slots between B-carrying ones so cumulative supply stays
  ahead of the PE. Evictions: vector+scalar engines alternate psum->sbuf
  casts; one/two SWDGE DMAs per slot; the final (S4D) slot packs live
  widths only and exits via HWDGE.

  Warm-up: ~10 dummy matmuls on zeroed SBUF during the initial DMA fill
  so the PE HAM clock-gate reaches 2.4 GHz before real work.

  MODE: "bf16" (default), "split3" (3x bf16 hi/lo matmuls, ~4.5e-6),
  "fp32" (exact, 4 cyc/row), or "f32r" (tf32-class, ~1.5e-4).
"""

import sys

sys.path.insert(0, "/opt/trn_rl_repo")

import numpy as np

N = 4096
N_CORES = 8
NB = N // 128
NJ = N // 512

MODE = "bf16"

# ---------------------------------------------------------------- schedule


def _enumerate_quads():
    """All real quads: (type, jg, band_a, band_b, r0).

    band [a,b] bk-range; quad rows bi in [r0, r0+3].
    """
    quads = []
    for jg in range(NJ):
        R = 4 * jg + 4
        # left-aligned bands of 8; trailing 4-band when R % 8 == 4
        bands = []
        a = 0
        while a < R:
            blen = 8 if R - a >= 8 else 4
            bands.append((a, a + blen - 1))
            a += blen
        for (a, b) in bands:
            diag = (b == R - 1)
            blen = b - a + 1
            # full quads: rows strictly above band
            for t in range(a // 4):
                if blen == 8:
                    quads.append(("F8D" if diag else "F8M", jg, a, b, 4 * t))
                else:
                    assert diag
                    quads.append(("F4D", jg, a, b, 4 * t))
            # staircase quads: rows inside the band
            if blen == 8:
                quads.append(("S8D" if diag else "S8M", jg, a, b, a))
                # lower staircase = 4-step stair over the band's last 4 bks
                quads.append(("S4D" if diag else "S4M", jg, a + 4, b, a + 4))
            else:
                quads.append(("S4D" if diag else "S4M", jg, a, b, a))
    return quads


# template slot types -> (steps, engagement, widths)
def _tmpl(ttype):
    L = 8 if "8" in ttype else 4
    stair = ttype.startswith("S")
    diag = ttype.endswith("D")
    widths = []
    for t in range(L):
        rem = L - 1 - t
        w = 512
        if diag and rem < 3:
            w = 128 * (rem + 1)
        widths.append(w)
    engage = [(0 if not stair else r) for r in range(4)]
    return L, engage, widths


# per-core slot list. Slots in _B_PAIRS are "B-sharing seconds": they ship
# only their A tracks and read the B columns from the partner slot's stream
# (both host quads of the SAME (jg, band), so the B data is identical).
_SLOT_TYPES = ["S8M", "S4M", "F8D", "F8D", "F8M", "F8M", "S8M", "S4M",
               "F4D", "S4D"]
_B_PAIRS = {1: 0, 3: 2, 5: 4, 7: 6}  # second_slot -> first_slot
# eviction DMA batches (consecutive slots share one large partition-major
# transfer; the final batch is the small S4D slot to keep the tail short)
_EVICT_BATCHES = [(0, 1, 2), (3, 4, 5), (6,), (7,), (8,), (9,)]


def _build_assignment():
    """assign[core][slot] = quad (type, jg, a, b, r0) hosted there.

    B-sharing pairs are co-located: (slot0, slot7) = one F8D band pair (or a
    same-band F4D pair), (slot3, slot8) = one F8M band pair, (slot1, slot2)
    and (slot5, slot6) = (S8x, S4x) stair pairs of the same band.
    """
    quads = _enumerate_quads()
    by_type = {}
    for q in quads:
        by_type.setdefault(q[0], []).append(q)
    for t in by_type:
        by_type[t].sort(key=lambda q: (q[1], q[2], q[4]))
    counts = {t: len(v) for t, v in by_type.items()}
    assert counts == {"F8M": 16, "F8D": 12, "F4D": 12, "S8M": 12,
                      "S8D": 4, "S4M": 12, "S4D": 8}, counts

    def pairs_of(lst):
        g = {}
        order = []
        for q in lst:
            k = (q[1], q[2])
            if k not in g:
                g[k] = []
                order.append(k)
            g[k].append(q)
        out = []
        for k in order:
            v = g[k]
            assert len(v) % 2 == 0, (k, len(v))
            for i in range(0, len(v), 2):
                out.append((v[i], v[i + 1]))
        return out

    f8m_pairs = pairs_of(by_type["F8M"])  # 8 pairs
    f8d_pairs = pairs_of(by_type["F8D"])  # 6 pairs
    f4d = by_type["F4D"]
    f4d_jg4 = [q for q in f4d if q[1] == 4][:2]
    f4d_jg6 = [q for q in f4d if q[1] == 6][:2]
    f8d_pairs += [tuple(f4d_jg4), tuple(f4d_jg6)]  # -> 8 pairs
    used = {id(q) for q in f4d_jg4 + f4d_jg6}
    f4d_rest = [q for q in f4d if id(q) not in used]  # 8 -> F4D slot
    assert len(f8m_pairs) == 8 and len(f8d_pairs) == 8 and len(f4d_rest) == 8

    # stair pairs: S4x of band (a..b) has a = band_a + 4 and partners the
    # S8x of the same band; trailing 4-bands have no S8 partner.
    s8_by_band = {}
    for q in by_type["S8M"] + by_type["S8D"]:
        s8_by_band[(q[1], q[2])] = q
    stair_pairs = []
    s4d_alone = []
    for q in by_type["S4M"] + by_type["S4D"]:
        p = s8_by_band.get((q[1], q[2] - 4))
        if p is not None:
            stair_pairs.append((p, q))
        else:
            s4d_alone.append(q)
    assert len(stair_pairs) == 16 and len(s4d_alone) == 4, (
        len(stair_pairs), len(s4d_alone))
    # mid pairs first, diag pairs last (sorted by partner type then band)
    stair_pairs.sort(key=lambda pq: (pq[0][0] != "S8M", pq[0][1], pq[0][2]))

    assign = [[None] * len(_SLOT_TYPES) for _ in range(N_CORES)]
    for c in range(N_CORES):
        assign[c][0], assign[c][1] = stair_pairs[2 * c]
        assign[c][2], assign[c][3] = f8d_pairs[c]
        assign[c][4], assign[c][5] = f8m_pairs[c]
        assign[c][6], assign[c][7] = stair_pairs[2 * c + 1]
        assign[c][8] = f4d_rest[c]
        assign[c][9] = s4d_alone[c] if c < 4 else None
    # paired slots must share (jg, band-end) so their B columns coincide
    for c in range(N_CORES):
        for s2, s1 in _B_PAIRS.items():
            qa, qb = assign[c][s1], assign[c][s2]
            assert qa is not None and qb is not None
            assert qa[1] == qb[1] and qa[3] == qb[3], (c, s1, s2, qa, qb)
    return assign


_ASSIGN = _build_assignment()
_TOTAL_STEPS = sum(_tmpl(t)[0] for t in _SLOT_TYPES)  # 60

_cache = {}


def _eff_w(w):
    if MODE == "f32r" and w < 256:
        return 256  # f32r runs at 1/4 rate below 256 cols
    return w


def _chunk_target(ci):
    """Per-chunk word budget: small first chunks so compute starts early."""
    return (160 * 1024, 384 * 1024)[ci] if ci < 2 else 704 * 1024


def _layout():
    """Variable-width per-step stream layout (template-static).

    Per step only the engaged A tracks and the live B columns are shipped.
    Element layout (au = A track unit cols, bu = B units):
      [A track 0 .. A track e-1 | B unit 0 (w cols) .. B unit bu-1]
    split3: au=256 (Ah|Al), bu=2 (Bh,Bl), bf16. fp32/f32r: au=128, bu=1, f32.

    Steps are packed into large chunks (one DMA each). Within a chunk the
    DRAM layout is partition-major [128, W_chunk]; each step occupies a
    column window.
    Returns (steps, chunks, total_words):
      steps[i]  = (e, w, au, bu, chunk_id, col_ofs, wpp)
      chunks[c] = (word_ofs, W)
    """
    au = 256 if MODE == "split3" else 128
    bu = 2 if MODE == "split3" else 1
    raw = []
    for s, ttype in enumerate(_SLOT_TYPES):
        L, engage, widths = _tmpl(ttype)
        has_b = s not in _B_PAIRS
        for t in range(L):
            e = sum(1 for r in range(4) if t >= engage[r])
            w = _eff_w(widths[t])
            wpp = au * e + (bu * w if has_b else 0)
            raw.append((s, t, e, w, au, bu, has_b, wpp))
    steps = []
    chunks = []
    word_ofs = 0
    col = 0
    for (s, t, e, w, au_, bu_, has_b, wpp) in raw:
        if col and 128 * (col + wpp) > _chunk_target(len(chunks)):
            chunks.append((word_ofs, col))
            word_ofs += 128 * col
            col = 0
        steps.append((s, t, e, w, au_, bu_, has_b, len(chunks), col, wpp))
        col += wpp
    chunks.append((word_ofs, col))
    total = word_ofs + 128 * col
    # index by (slot, t) for B-sharing partner lookup; widths must agree at
    # the aligned step (t_partner = t + L_partner - L_second)
    idx = {(s, t): rec for rec in steps for (s, t) in [(rec[0], rec[1])]}
    for s2, s1 in _B_PAIRS.items():
        L2 = _tmpl(_SLOT_TYPES[s2])[0]
        L1 = _tmpl(_SLOT_TYPES[s1])[0]
        for t in range(L2):
            r2 = idx[(s2, t)]
            r1 = idx[(s1, t + L1 - L2)]
            assert r1[3] == r2[3], (s2, t, r1[3], r2[3])  # same w
    return steps, chunks, total, idx

# ------------------------------------------------------------------ device


def _build_nc():
    import concourse.bacc as bacc
    import concourse.mybir as mybir
    import concourse.tile as tile

    f32 = mybir.dt.float32
    nc = bacc.Bacc()
    if MODE in ("split3", "bf16"):
        s_dt = mybir.dt.bfloat16
        store_dt = mybir.dt.bfloat16
    else:
        s_dt = {"fp32": mybir.dt.float32, "f32r": mybir.dt.float32r}[MODE]
        store_dt = mybir.dt.float32
    # bf16 mode evicts partials in bf16 (halves output DMA; host upconverts)
    cp_dt = mybir.dt.bfloat16 if MODE == "bf16" else f32
    steps_layout, chunks, total_words, step_idx = _layout()
    s_in = nc.declare_dram_parameter("S", [total_words], store_dt,
                                     isOutput=False)
    # partition-major across slots: batched eviction DMAs get long
    # per-partition contiguous runs (better descriptor efficiency)
    cp = nc.declare_dram_parameter("CP", [128, len(_SLOT_TYPES) * 2048],
                                   cp_dt, isOutput=True)

    with tile.TileContext(nc) as tc:
        with (
            tc.tile_pool(name="st", bufs=1) as s_pool,
            tc.tile_pool(name="co", bufs=1) as c_pool,
            tc.tile_pool(name="ps", bufs=2, space="PSUM") as ps_pool,
        ):
            # PE warm-up: dummy matmuls on zeroed SBUF spanning the initial
            # DMA wait so HAM un-throttles (1.2->2.4 GHz) before real work.
            wz = s_pool.tile([128, 512], s_dt, tag="wz", name="wz")
            nc.vector.memset(wz[:], 0)
            wu = ps_pool.tile([128, 512], f32, tag="p0", name="wu")
            for i in range(14):
                nc.tensor.matmul(wu[:], lhsT=wz[:, :128], rhs=wz[:],
                                 start=True, stop=True)
            # one big input DMA per chunk; steps slice column windows
            ch_tiles = []
            for ci, (ofs, W) in enumerate(chunks):
                src = s_in[ofs:ofs + 128 * W] \
                    .rearrange("(p w) -> p w", p=128).bitcast(s_dt)
                ch = s_pool.tile([128, W], s_dt, tag=f"ch{ci}",
                                 name=f"ch_{ci}")
                nc.sync.dma_start(out=ch[:], in_=src)
                ch_tiles.append(ch)
            cursor = 0
            batch_i = 0
            cb = None
            cb_base = 0
            for s, ttype in enumerate(_SLOT_TYPES):
                L, engage, widths = _tmpl(ttype)
                ps = [
                    ps_pool.tile([128, 512], f32, tag=f"p{r}",
                                 name=f"ps_{s}_{r}")
                    for r in range(4)
                ]
                for t in range(L):
                    _s, _t, e, w, au, bu, has_b, ci, col, wpp = \
                        steps_layout[cursor]
                    oc = 512 - w
                    st = ch_tiles[ci][:, col:col + wpp]
                    if has_b:
                        b_tile, b_ofs = st, au * e
                    else:
                        s1 = _B_PAIRS[s]
                        L1 = _tmpl(_SLOT_TYPES[s1])[0]
                        r1 = step_idx[(s1, t + L1 - L)]
                        e1, ci1, col1 = r1[2], r1[7], r1[8]
                        b_tile = ch_tiles[ci1][:, col1:col1 + r1[9]]
                        b_ofs = au * e1
                    for r in range(4):
                        if t < engage[r]:
                            continue
                        first = (t == engage[r])
                        last = (t == L - 1)
                        if MODE == "split3":
                            ah = st[:, au * r:au * r + 128]
                            al = st[:, au * r + 128:au * (r + 1)]
                            bh = b_tile[:, b_ofs:b_ofs + w]
                            bl = b_tile[:, b_ofs + w:b_ofs + 2 * w]
                            nc.tensor.matmul(ps[r][:, oc:], lhsT=ah, rhs=bh,
                                             start=first, stop=False)
                            nc.tensor.matmul(ps[r][:, oc:], lhsT=al, rhs=bh,
                                             start=False, stop=False)
                            nc.tensor.matmul(ps[r][:, oc:], lhsT=ah, rhs=bl,
                                             start=False, stop=last)
                        else:
                            nc.tensor.matmul(
                                ps[r][:, oc:],
                                lhsT=st[:, au * r:au * (r + 1)],
                                rhs=b_tile[:, b_ofs:b_ofs + w],
                                start=first, stop=last,
                            )
                    cursor += 1
                base = 2048 * s
                c_t = c_pool.tile([128, 2048], cp_dt, tag=f"c{s}",
                                  name=f"c_{s}")
                # early-slot evictions queue on the sync HWDGE ring BEHIND
                # all input chunks: they drain after input finishes (the
                # c_t tiles are never reused, so late completion is free)
                # and stop stealing mid-stream HBM bandwidth from input.
                # Tail slots stay on the idle SWDGE ring for prompt exit.
                ev = nc.sync if s < 8 else nc.gpsimd
                if ttype == "S4D":
                    # live widths per row are 512,384,256,128 -> pack tight
                    ofs = 0
                    for r in range(4):
                        wv = 512 - 128 * r
                        dst = c_t[:, ofs:ofs + wv]
                        src = ps[r][:, 512 - wv:]
                        if r % 2 == 0:
                            nc.vector.tensor_copy(dst, src)
                        else:
                            nc.scalar.copy(dst, src)
                        ofs += wv
                    ev.dma_start(out=cp[:, base:base + ofs],
                                 in_=c_t[:, :ofs])
                else:
                    for r in range(4):
                        dst = c_t[:, 512 * r:512 * (r + 1)]
                        if r % 2 == 0:
                            nc.vector.tensor_copy(dst, ps[r][:])
                        else:
                            nc.scalar.copy(dst, ps[r][:])
                        if r % 2 == 1:
                            h = r // 2
                            ev.dma_start(
                                out=cp[:, base + 1024 * h:
                                       base + 1024 * (h + 1)],
                                in_=c_t[:, 1024 * h:1024 * (h + 1)])
            assert cursor == _TOTAL_STEPS
    nc.finalize()
    return nc


def get_nc():
    key = ("nc", MODE)
    if key not in _cache:
        _cache[key] = _build_nc()
    return _cache[key]


# ------------------------------------------------------------------- host


def _make_blocks(A, B):
    """Mode-specific block views for packing."""
    A4 = A.reshape(NB, 128, NB, 128).transpose(0, 2, 3, 1)
    B4 = B.reshape(NB, 128, NJ, 512).transpose(0, 2, 1, 3)
    if MODE == "bf16":
        import ml_dtypes

        bf = ml_dtypes.bfloat16
        return {"A": [A4.astype(bf)], "B": [B4.astype(bf)], "dtype": bf}
    if MODE != "split3":
        return {"A": [A4], "B": [B4], "dtype": np.float32}
    import ml_dtypes

    bf = ml_dtypes.bfloat16
    A4h = A4.astype(bf)
    A4l = (A4 - A4h.astype(np.float32)).astype(bf)
    B4h = B4.astype(bf)
    B4l = (B4 - B4h.astype(np.float32)).astype(bf)
    return {"A": [A4h, A4l], "B": [B4h, B4l], "dtype": bf}


def _pack_core(c, blocks):
    """Flat variable-width S stream for core c (layout per _layout()).

    A blocks are transposed ([p,m] = A[128bi+m, 128bk+p]); B blocks are
    128x512 (only the live [oc:] columns are shipped).
    """
    steps_layout, chunks, total_words, _ = _layout()
    chs = [np.zeros((128, W), dtype=blocks["dtype"]) for _, W in chunks]
    cursor = 0
    for s, ttype in enumerate(_SLOT_TYPES):
        L, engage, widths = _tmpl(ttype)
        q = _ASSIGN[c][s]
        if q is None:  # ghost slot: leave zeros
            cursor += L
            continue
        qtype, jg, a, b, r0 = q
        base = b - L + 1  # bk at template step 0 (right-aligned hosting)
        for t in range(L):
            _s, _t, e, w, au, bu, has_b, ci, col, wpp = steps_layout[cursor]
            bk = base + t
            row = chs[ci][:, col:col + wpp]
            oc = 512 - w
            if has_b and bk >= a:
                for h in range(bu):
                    row[:, au * e + w * h:au * e + w * (h + 1)] = \
                        blocks["B"][h][bk, jg][:, oc:]
            for r in range(e):
                bi = r0 + r
                if bk >= a and bk >= bi:
                    for h in range(len(blocks["A"])):
                        row[:, au * r + 128 * h:au * r + 128 * (h + 1)] = \
                            blocks["A"][h][bi, bk]
            cursor += 1
    return np.concatenate([ch.reshape(-1) for ch in chs])


def _out_np_dtype():
    if MODE == "bf16":
        import ml_dtypes

        return ml_dtypes.bfloat16
    return np.float32


def _get_runner():
    """Build (once per process/MODE) a cached jitted SPMD executable.

    Mirrors bass2jax.run_bass_via_pjrt's multi-core path, but reuses the
    compiled executable across kernel() calls.
    """
    key = ("runner", MODE)
    if key in _cache:
        return _cache[key]
    import jax
    from jax.sharding import Mesh, PartitionSpec
    from jax.experimental.shard_map import shard_map
    from concourse import bass2jax, mybir

    nc = get_nc()
    bass2jax.install_neuronx_cc_hook()
    partition_name = (nc.partition_id_tensor.name
                      if nc.partition_id_tensor else None)
    out_shape = (128, len(_SLOT_TYPES) * 2048)
    out_aval = jax.core.ShapedArray(out_shape, _out_np_dtype())
    in_names = ["S", "CP"]
    if partition_name is not None:
        in_names.append(partition_name)

    def _body(s_arr, zeros):
        operands = [s_arr, zeros]
        if partition_name is not None:
            operands.append(bass2jax.partition_id_tensor())
        outs = bass2jax._bass_exec_p.bind(
            *operands, out_avals=(out_aval,), in_names=tuple(in_names),
            out_names=("CP",), lowering_input_output_aliases=(),
            sim_require_finite=True, sim_require_nnan=True, nc=nc)
        return outs[0]

    devices = jax.devices()[:N_CORES]
    mesh = Mesh(np.asarray(devices), ("core",))
    sharded = jax.jit(
        shard_map(_body, mesh=mesh,
                  in_specs=(PartitionSpec("core"),) * 2,
                  out_specs=PartitionSpec("core"), check_rep=False),
        donate_argnums=(1,), keep_unused=True)
    _cache[key] = sharded
    return sharded


def kernel(A: np.ndarray, B: np.ndarray) -> np.ndarray:
    A = np.asarray(A, dtype=np.float32)
    B = np.asarray(B, dtype=np.float32)

    blocks = _make_blocks(A, B)
    s_all = np.concatenate([_pack_core(c, blocks) for c in range(N_CORES)],
                           axis=0)
    zeros = np.zeros((N_CORES * 128, len(_SLOT_TYPES) * 2048),
                     _out_np_dtype())
    runner = _get_runner()
    out = np.asarray(runner(s_all, zeros))
    per_core = out.reshape(N_CORES, 128, len(_SLOT_TYPES) * 2048)
    if per_core.dtype != np.float32:
        per_core = per_core.astype(np.float32)

    C = np.zeros((N, N), dtype=np.float32)
    for c in range(N_CORES):
        cpk = per_core[c]
        for s, ttype in enumerate(_SLOT_TYPES):
            q = _ASSIGN[c][s]
            if q is None:
                continue
            qtype, jg, a, b, r0 = q
            if ttype == "S4D":
                ofs = 2048 * s
                for r in range(4):
                    bi = r0 + r
                    wv = 512 - 128 * r
                    C[128 * bi:128 * (bi + 1),
                      512 * jg + 128 * r:512 * (jg + 1)] += \
                        cpk[:, ofs:ofs + wv]
                    ofs += wv
                continue
            for r in range(4):
                bi = r0 + r
                blk = cpk[:, 2048 * s + 512 * r:2048 * s + 512 * (r + 1)]
                # written psum region starts at the track's start width
                L, engage, widths = _tmpl(ttype)
                w0 = widths[engage[r]]
                if MODE == "f32r" and w0 < 256:
                    w0 = 256
                lo = 512 - w0
                C[128 * bi:128 * (bi + 1),
                  512 * jg + lo:512 * (jg + 1)] += blk[:, lo:]
    return C


def _make_in_maps(A, B):
    A = np.asarray(A, dtype=np.float32)
    B = np.asarray(B, dtype=np.float32)
    blocks = _make_blocks(A, B)
    return [{"S": _pack_core(c, blocks)} for c in range(N_CORES)]

